# revision 12
# baseline (speedup 1.0000x reference)
"""PaiNN message-passing kernel for Trainium2, 8-way SPMD.

Strategy:
 - Host: bin-pack nodes into (ncores*nblk) destination blocks (<=128 nodes,
   <=blk_e edges each), balance blocks across cores. Edges are grouped by
   destination block; per-edge sinc radial basis (with cosine cutoff and the
   rbf bias folded in as an extra basis row) is precomputed on host.
 - Device (per core, same program, different data):
     Phase A: phi table = (silu(node_scalar @ W1 + b1) @ W2 + b2) for all
              nodes, bf16, written to a DRAM scratch table (per-node reuse:
              each node's phi is consumed by ~16 edges).
     Phase B: per 2048-edge block: indirect-DMA gather phi[src], nv[src];
              rbf = sincT_ext @ Wr_ext (per 128-edge chunk, tensor engine);
              pre = phi_gather * rbf;  one-hot(dst_local) built by iota
              compare;  scatter-add via one-hot matmuls accumulated in PSUM
              over the block's 16 chunks:
                psA[d, 0:128]   += oh^T @ s2            (delta_s)
                psA[d, 128:512] += oh^T @ (nv * s1)     (delta_v term 1)
                psB[d, (c,f)]   += oh^T @ (rhat x s3)   (delta_v term 2)
              flush: add node bases, DMA out.
 - Host: unpermute block rows back to node order.
"""

import sys

sys.path.insert(0, "/opt/trn_rl_repo")

from contextlib import ExitStack
from dataclasses import dataclass

import numpy as np
import ml_dtypes

import concourse.bass as bass
import concourse.bacc as bacc
import concourse.tile as tile
from concourse import mybir
from concourse.bass import IndirectOffsetOnAxis

BF16 = ml_dtypes.bfloat16
AF = mybir.ActivationFunctionType


@dataclass(frozen=True)
class Cfg:
    n: int = 50000       # nodes
    e: int = 800000      # edges
    f: int = 128         # features
    nrbf: int = 20
    cutoff: float = 5.0
    ncores: int = 8
    nblk: int = 50       # destination blocks per core
    tpb: int = 4         # 512-edge macrotiles per block
    ntab: int = 50176    # padded table rows (multiple of 512, >= n)

    @property
    def blk_e(self):  # edges per block (padded)
        return self.tpb * 512

    @property
    def chunks(self):  # 128-edge chunks per block
        return self.tpb * 4

    @property
    def npad(self):  # node slots per core
        return self.nblk * 128


CFG = Cfg()
G3 = 384  # 3 * f


def build_program(cfg: Cfg, debug: bool = False):
    """Build the single-core Bass program (same for all cores)."""
    nc = bacc.Bacc()
    fdt = mybir.dt.float32
    bdt = mybir.dt.bfloat16
    idt = mybir.dt.int32

    f = cfg.f
    nblk, tpb, chunks = cfg.nblk, cfg.tpb, cfg.chunks

    # ---- inputs ----
    nsT = nc.dram_tensor("nsT", [f, cfg.ntab], bdt, kind="ExternalInput")
    nv_tab = nc.dram_tensor("nv_tab", [cfg.ntab, G3], bdt, kind="ExternalInput")
    w1 = nc.dram_tensor("w1", [f, f], bdt, kind="ExternalInput")
    b1 = nc.dram_tensor("b1", [f, 1], fdt, kind="ExternalInput")
    w2 = nc.dram_tensor("w2", [f, G3], bdt, kind="ExternalInput")
    b2row = nc.dram_tensor("b2row", [1, G3], bdt, kind="ExternalInput")
    wr_ext = nc.dram_tensor("wr_ext", [24, G3], bdt, kind="ExternalInput")
    iota4 = nc.dram_tensor("iota4", [128, 512], fdt, kind="ExternalInput")
    ones1 = nc.dram_tensor("ones1", [1, 128], bdt, kind="ExternalInput")
    srcidx = nc.dram_tensor("srcidx", [128, nblk * chunks], idt, kind="ExternalInput")
    dstloc = nc.dram_tensor("dstloc", [128, nblk * chunks], fdt, kind="ExternalInput")
    rhat = nc.dram_tensor("rhat", [128, nblk * chunks * 3], bdt, kind="ExternalInput")
    sincT = nc.dram_tensor("sincT", [24, nblk * cfg.blk_e], bdt, kind="ExternalInput")
    ns_base = nc.dram_tensor("ns_base", [cfg.npad, f], fdt, kind="ExternalInput")
    nv_base = nc.dram_tensor("nv_base", [cfg.npad, G3], fdt, kind="ExternalInput")

    # ---- outputs ----
    out_s = nc.dram_tensor("out_s", [cfg.npad, f], fdt, kind="ExternalOutput")
    out_v = nc.dram_tensor("out_v", [cfg.npad, G3], fdt, kind="ExternalOutput")
    if debug:
        dbg = {
            "dbg_phi": nc.dram_tensor("dbg_phi", [128, G3], fdt, kind="ExternalOutput"),
            "dbg_phig": nc.dram_tensor("dbg_phig", [128, chunks * G3], fdt, kind="ExternalOutput"),
            "dbg_nvg": nc.dram_tensor("dbg_nvg", [128, chunks * G3], fdt, kind="ExternalOutput"),
            "dbg_rbf": nc.dram_tensor("dbg_rbf", [128, 4 * G3], fdt, kind="ExternalOutput"),
            "dbg_pre": nc.dram_tensor("dbg_pre", [128, 4 * G3], fdt, kind="ExternalOutput"),
            "dbg_oh": nc.dram_tensor("dbg_oh", [128, 512], fdt, kind="ExternalOutput"),
            "dbg_srhs": nc.dram_tensor("dbg_srhs", [128, 2048], fdt, kind="ExternalOutput"),
            "dbg_brhs": nc.dram_tensor("dbg_brhs", [128, 4 * G3], fdt, kind="ExternalOutput"),
            "dbg_psA": nc.dram_tensor("dbg_psA", [128, 512], fdt, kind="ExternalOutput"),
            "dbg_psB": nc.dram_tensor("dbg_psB", [128, G3], fdt, kind="ExternalOutput"),
        }

    with tile.TileContext(nc) as tc, ExitStack() as ctx:
        const = ctx.enter_context(tc.tile_pool(name="const", bufs=1))

        # constants / whole small inputs -> SBUF once
        w1_sb = const.tile([f, f], bdt)
        nc.sync.dma_start(out=w1_sb[:], in_=w1[:])
        b1_sb = const.tile([f, 1], fdt)
        nc.sync.dma_start(out=b1_sb[:], in_=b1[:])
        w2_sb = const.tile([f, G3], bdt)
        nc.sync.dma_start(out=w2_sb[:], in_=w2[:])
        b2_sb = const.tile([1, G3], bdt)
        nc.sync.dma_start(out=b2_sb[:], in_=b2row[:])
        wr_sb = const.tile([24, G3], bdt)
        nc.sync.dma_start(out=wr_sb[:], in_=wr_ext[:])
        iota_sb = const.tile([128, 512], fdt)
        nc.sync.dma_start(out=iota_sb[:], in_=iota4[:])
        ones_sb = const.tile([1, 128], bdt)
        nc.sync.dma_start(out=ones_sb[:], in_=ones1[:])
        src_sb = const.tile([128, nblk * chunks], idt)
        nc.sync.dma_start(out=src_sb[:], in_=srcidx[:])
        dst_sb = const.tile([128, nblk * chunks], fdt)
        nc.sync.dma_start(out=dst_sb[:], in_=dstloc[:])
        rhat_sb = const.tile([128, nblk * chunks * 3], bdt)
        nc.sync.dma_start(out=rhat_sb[:], in_=rhat[:])

        # DRAM scratch: combined [phi | nv] table, 768 values per node row
        dram = ctx.enter_context(tc.tile_pool(name="dram", bufs=1, space="DRAM"))
        comb_tab = dram.tile([cfg.ntab, 2 * G3], bdt)

        # ---- Phase A: phi table ----
        with tc.tile_pool(name="pre_sb", bufs=3) as psb, \
             tc.tile_pool(name="pre_ps", bufs=2, space="PSUM") as pps:
            for o in range(0, cfg.ntab, 512):
                w = min(512, cfg.ntab - o)
                nst_t = psb.tile([f, 512], bdt, tag="nst")
                nc.sync.dma_start(out=nst_t[:, :w], in_=nsT[:, o:o + w])
                h_ps = pps.tile([f, 512], fdt, tag="hps")
                nc.tensor.matmul(h_ps[:, :w], lhsT=w1_sb[:], rhs=nst_t[:, :w],
                                 start=True, stop=True)
                hb_sb = psb.tile([f, 512], fdt, tag="hbsb")
                nc.scalar.activation(hb_sb[:, :w], h_ps[:, :w], AF.Identity,
                                     bias=b1_sb[:], scale=1.0)
                sg_sb = psb.tile([f, 512], fdt, tag="sgsb")
                nc.scalar.activation(sg_sb[:, :w], h_ps[:, :w], AF.Sigmoid,
                                     bias=b1_sb[:], scale=1.0)
                h_sb = psb.tile([f, 512], bdt, tag="hsb")
                nc.vector.tensor_tensor(out=h_sb[:, :w], in0=hb_sb[:, :w],
                                        in1=sg_sb[:, :w],
                                        op=mybir.AluOpType.mult)
                for c in range(0, w, 128):
                    cw = min(128, w - c)
                    phi_ps = pps.tile([128, G3], fdt, tag="phips")
                    nc.tensor.matmul(phi_ps[:cw, :], lhsT=h_sb[:, c:c + cw],
                                     rhs=w2_sb[:], start=True, stop=False)
                    nc.tensor.matmul(phi_ps[:cw, :], lhsT=ones_sb[:, :cw],
                                     rhs=b2_sb[:], start=False, stop=True)
                    phi_sb = psb.tile([128, 2 * G3], bdt, tag="phisb")
                    nc.scalar.copy(phi_sb[:cw, :G3], phi_ps[:cw, :])
                    nc.sync.dma_start(out=phi_sb[:cw, G3:],
                                      in_=nv_tab[o + c:o + c + cw, :])
                    nc.sync.dma_start(out=comb_tab[o + c:o + c + cw, :],
                                      in_=phi_sb[:cw, :])

        # Phase A writes phi_tab via DMA; the indirect gathers below read it.
        # DRAM RAW deps are not reliably tracked -> hard fence.
        tc.strict_bb_all_engine_barrier()

        # ---- Phase B: edge blocks ----
        with tc.tile_pool(name="gath", bufs=2) as gpool, \
             tc.tile_pool(name="work", bufs=3) as wpool, \
             tc.tile_pool(name="mps", bufs=3, space="PSUM") as mps, \
             tc.tile_pool(name="acc", bufs=2, space="PSUM") as acc, \
             tc.tile_pool(name="flush", bufs=2) as fpool:
            for b in range(nblk):
                ecol = b * chunks          # chunk-column base for this block
                def dump(dst_dram, src_ap, cols):
                    t = fpool.tile([128, cols], fdt, tag="dbgt", name="dbgt")
                    nc.scalar.copy(t[:, :cols], src_ap)
                    nc.sync.dma_start(out=dst_dram[:], in_=t[:, :cols])

                # gather [phi | nv] rows per chunk (128 rows / instr:
                # HW indirect DMA honors one offset per partition)
                comb_g = gpool.tile([128, chunks * 2 * G3], bdt, tag="combg")
                for ch in range(chunks):
                    nc.gpsimd.indirect_dma_start(
                        out=comb_g[:, ch * 2 * G3:(ch + 1) * 2 * G3],
                        out_offset=None, in_=comb_tab[:],
                        in_offset=IndirectOffsetOnAxis(
                            ap=src_sb[:, ecol + ch:ecol + ch + 1], axis=0))
                if debug and b == 0:
                    pt = fpool.tile([128, 2 * G3], bdt, name="pt")
                    nc.sync.dma_start(out=pt[:], in_=comb_tab[0:128, :])
                    dump(dbg["dbg_phi"], pt[:, :G3], G3)
                    dump(dbg["dbg_phig"],
                         bass.AP(comb_g.tensor, comb_g.offset,
                                 [comb_g.ap[0], [2 * G3, chunks], [1, G3]]),
                         chunks * G3)
                    dump(dbg["dbg_nvg"],
                         bass.AP(comb_g.tensor, comb_g[:, G3].offset,
                                 [comb_g.ap[0], [2 * G3, chunks], [1, G3]]),
                         chunks * G3)
                sinc_t = gpool.tile([24, cfg.blk_e], bdt, tag="sinct")
                nc.sync.dma_start(out=sinc_t[:],
                                  in_=sincT[:, b * cfg.blk_e:(b + 1) * cfg.blk_e])

                psA = acc.tile([128, 512], fdt, tag="psA")
                psB = acc.tile([128, G3], fdt, tag="psB")

                for m in range(tpb):  # 512-edge macrotiles
                    mc = 4 * m
                    # one-hot compare: oh4[p, j*128+d] = (dstloc[p, j] == d)
                    oh4 = wpool.tile([128, 512], bdt, tag="oh4")
                    nc.vector.tensor_tensor(
                        out=oh4[:],
                        in0=bass.AP(dst_sb.tensor, dst_sb[:, ecol + mc].offset,
                                    [dst_sb.ap[0], [1, 4], [0, 128]]),
                        in1=bass.AP(iota_sb.tensor, iota_sb.offset,
                                    [iota_sb.ap[0], [128, 4], [1, 128]]),
                        op=mybir.AluOpType.is_equal)

                    # rbf per chunk -> psum, then copy to bf16 sbuf
                    rbf_sb = wpool.tile([128, 4 * G3], bdt, tag="rbfsb")
                    for j in range(4):
                        ch = mc + j
                        rbf_ps = mps.tile([128, G3], fdt, tag="rbfps")
                        nc.tensor.matmul(
                            rbf_ps[:],
                            lhsT=sinc_t[:, ch * 128:(ch + 1) * 128],
                            rhs=wr_sb[:], start=True, stop=True)
                        nc.scalar.copy(rbf_sb[:, j * G3:(j + 1) * G3], rbf_ps[:])

                    if debug and b == 0 and m == 0:
                        dump(dbg["dbg_oh"], oh4[:], 512)
                        dump(dbg["dbg_rbf"], rbf_sb[:], 4 * G3)
                    # pre = phi_gather * rbf  (bf16)
                    pre_mt = wpool.tile([128, 4 * G3], bdt, tag="premt")
                    nc.vector.tensor_tensor(
                        out=pre_mt[:],
                        in0=bass.AP(comb_g.tensor,
                                    comb_g[:, mc * 2 * G3].offset,
                                    [comb_g.ap[0], [2 * G3, 4], [1, G3]]),
                        in1=rbf_sb[:], op=mybir.AluOpType.mult)

                    if debug and b == 0 and m == 0:
                        dump(dbg["dbg_pre"], pre_mt[:], 4 * G3)
                    # scatter rhs: [s2 | nv*s1] per chunk (512 cols per chunk)
                    scat_rhs = wpool.tile([128, 4 * 512], bdt, tag="scatrhs")
                    nc.vector.tensor_copy(  # s2 -> cols 0:128
                        out=bass.AP(scat_rhs.tensor, scat_rhs.offset,
                                    [scat_rhs.ap[0], [512, 4], [1, 128]]),
                        in_=bass.AP(pre_mt.tensor, pre_mt[:, 128].offset,
                                    [pre_mt.ap[0], [G3, 4], [1, 128]]))
                    nc.vector.tensor_tensor(  # nv * s1 -> cols 128:512
                        out=bass.AP(scat_rhs.tensor, scat_rhs[:, 128].offset,
                                    [scat_rhs.ap[0], [512, 4], [3, 128], [1, 3]]),
                        in0=bass.AP(comb_g.tensor,
                                    comb_g[:, mc * 2 * G3 + G3].offset,
                                    [comb_g.ap[0], [2 * G3, 4], [3, 128], [1, 3]]),
                        in1=bass.AP(pre_mt.tensor, pre_mt.offset,
                                    [pre_mt.ap[0], [G3, 4], [1, 128], [0, 3]]),
                        op=mybir.AluOpType.mult)

                    if debug and b == 0 and m == 0:
                        dump(dbg["dbg_srhs"], scat_rhs[:], 2048)
                    # brhs = rhat x s3, layout (c, f) per chunk
                    brhs = wpool.tile([128, 4 * G3], bdt, tag="brhs")
                    nc.vector.tensor_tensor(
                        out=bass.AP(brhs.tensor, brhs.offset,
                                    [brhs.ap[0], [G3, 4], [128, 3], [1, 128]]),
                        in0=bass.AP(rhat_sb.tensor,
                                    rhat_sb[:, (ecol + mc) * 3].offset,
                                    [rhat_sb.ap[0], [3, 4], [1, 3], [0, 128]]),
                        in1=bass.AP(pre_mt.tensor, pre_mt[:, 256].offset,
                                    [pre_mt.ap[0], [G3, 4], [0, 3], [1, 128]]),
                        op=mybir.AluOpType.mult)

                    if debug and b == 0 and m == 0:
                        dump(dbg["dbg_brhs"], brhs[:], 4 * G3)
                    for j in range(4):
                        ch = mc + j
                        first = ch == 0
                        last = ch == chunks - 1
                        nc.tensor.matmul(
                            psA[:], lhsT=oh4[:, j * 128:(j + 1) * 128],
                            rhs=scat_rhs[:, j * 512:(j + 1) * 512],
                            start=first, stop=last)
                        nc.tensor.matmul(
                            psB[:], lhsT=oh4[:, j * 128:(j + 1) * 128],
                            rhs=brhs[:, j * G3:(j + 1) * G3],
                            start=first, stop=last)

                if debug and b == 0:
                    dump(dbg["dbg_psA"], psA[:], 512)
                    dump(dbg["dbg_psB"], psB[:], G3)
                # flush block b
                bs = fpool.tile([128, f], fdt, tag="bs")
                nc.sync.dma_start(out=bs[:], in_=ns_base[b * 128:(b + 1) * 128, :])
                bv = fpool.tile([128, G3], fdt, tag="bv")
                nc.sync.dma_start(out=bv[:], in_=nv_base[b * 128:(b + 1) * 128, :])
                os_sb = fpool.tile([128, f], fdt, tag="ossb")
                nc.vector.tensor_tensor(out=os_sb[:], in0=psA[:, 0:128],
                                        in1=bs[:], op=mybir.AluOpType.add)
                ov_sb = fpool.tile([128, G3], fdt, tag="ovsb")
                nc.vector.tensor_tensor(out=ov_sb[:], in0=psA[:, 128:512],
                                        in1=bv[:], op=mybir.AluOpType.add)
                nc.vector.tensor_tensor(  # += psB with (c,f)->(f,c) reorder
                    out=bass.AP(ov_sb.tensor, ov_sb.offset,
                                [ov_sb.ap[0], [1, 3], [3, 128]]),
                    in0=bass.AP(ov_sb.tensor, ov_sb.offset,
                                [ov_sb.ap[0], [1, 3], [3, 128]]),
                    in1=bass.AP(psB.tensor, psB.offset,
                                [psB.ap[0], [128, 3], [1, 128]]),
                    op=mybir.AluOpType.add)
                nc.sync.dma_start(out=out_s[b * 128:(b + 1) * 128, :], in_=os_sb[:])
                nc.sync.dma_start(out=out_v[b * 128:(b + 1) * 128, :], in_=ov_sb[:])

    nc.compile()
    return nc


# ---------------------------------------------------------------------------
# host-side preparation
# ---------------------------------------------------------------------------

def pack_blocks(dst: np.ndarray, cfg: Cfg):
    """Assign nodes to (ncores*nblk) blocks, <=128 nodes & <=blk_e edges each,
    then blocks to cores. Returns per-core list of node-id arrays."""
    import heapq

    nbins = cfg.ncores * cfg.nblk
    deg = np.bincount(dst, minlength=cfg.n)
    order = np.argsort(-deg, kind="stable")
    bin_nodes = [[] for _ in range(nbins)]
    bin_edges = np.zeros(nbins, dtype=np.int64)
    heap = [(0, b) for b in range(nbins)]
    heapq.heapify(heap)
    spill = []
    for nid in order:
        d = int(deg[nid])
        while True:
            e, b = heapq.heappop(heap)
            if len(bin_nodes[b]) < 128 and e + d <= cfg.blk_e:
                bin_nodes[b].append(nid)
                bin_edges[b] = e + d
                heapq.heappush(heap, (e + d, b))
                break
            spill.append((e, b))
            if not heap:
                raise RuntimeError("bin packing failed")
        for item in spill:
            heapq.heappush(heap, item)
        spill.clear()

    # blocks -> cores, balanced by edge count
    order_b = np.argsort(-bin_edges, kind="stable")
    core_load = [(0, k, 0) for k in range(cfg.ncores)]  # (edges, core, nblocks)
    heapq.heapify(core_load)
    core_blocks = [[] for _ in range(cfg.ncores)]
    pending = []
    for b in order_b:
        while True:
            e, k, cnt = heapq.heappop(core_load)
            if cnt < cfg.nblk:
                core_blocks[k].append(b)
                heapq.heappush(core_load, (e + int(bin_edges[b]), k, cnt + 1))
                break
            pending.append((e, k, cnt))
        for item in pending:
            heapq.heappush(core_load, item)
        pending.clear()
    return [[np.asarray(bin_nodes[b], dtype=np.int64) for b in blks]
            for blks in core_blocks]


def prep_core_inputs(cfg: Cfg, core_nodes, node_scalar, node_vector, r_vec,
                     W1, b1, W2, b2, Wr, br, src, dst, edge_order, node_ptr):
    """Build the in_map for one core. core_nodes: list of nblk node-id arrays.
    edge_order: edge ids sorted by dst; node_ptr: CSR offsets into it."""
    f, G = cfg.f, G3
    nblk, chunks, blk_e = cfg.nblk, cfg.chunks, cfg.blk_e

    srccols = np.zeros((128, nblk * chunks), dtype=np.int32)
    dstcols = np.zeros((128, nblk * chunks), dtype=np.float32)
    rhatcols = np.zeros((128, nblk * chunks * 3), dtype=np.float32)
    sincT = np.zeros((24, nblk * blk_e), dtype=np.float32)
    ns_base = np.zeros((cfg.npad, f), dtype=np.float32)
    nv_base = np.zeros((cfg.npad, G), dtype=np.float32)
    perm_nodes = np.full(cfg.npad, -1, dtype=np.int64)

    for b, nodes in enumerate(core_nodes):
        nb = len(nodes)
        perm_nodes[b * 128:b * 128 + nb] = nodes
        ns_base[b * 128:b * 128 + nb] = node_scalar[nodes]
        nv_base[b * 128:b * 128 + nb] = node_vector[nodes].reshape(nb, G)
        # edges of this block, with dst_local
        eids = np.concatenate([edge_order[node_ptr[n]:node_ptr[n + 1]]
                               for n in nodes]) if nb else np.empty(0, np.int64)
        dl = np.repeat(np.arange(nb),
                       [node_ptr[n + 1] - node_ptr[n] for n in nodes])
        ne = len(eids)
        assert ne <= blk_e, f"block overflow {ne}"
        esrc = src[eids]
        rv = r_vec[eids]
        d = np.sqrt((rv * rv).sum(1))
        fc = np.where(d < cfg.cutoff,
                      0.5 * (np.cos(np.pi * d / cfg.cutoff) + 1.0), 0.0)
        nvals = np.arange(1, cfg.nrbf + 1, dtype=np.float64)
        sinc = np.sin(d[:, None] * nvals * (np.pi / cfg.cutoff)) / d[:, None]
        rh = rv / d[:, None]
        # chunk-major layout: edge t -> chunk j=t//128, partition p=t%128
        j = np.arange(ne) // 128
        p = np.arange(ne) % 128
        cbase = b * chunks + j
        srccols[p, cbase] = esrc
        dstcols[p, cbase] = dl
        rhatcols[p, cbase * 3 + 0] = rh[:, 0]
        rhatcols[p, cbase * 3 + 1] = rh[:, 1]
        rhatcols[p, cbase * 3 + 2] = rh[:, 2]
        ecols = b * blk_e + np.arange(ne)
        sincT[:cfg.nrbf, ecols] = (sinc * fc[:, None]).T
        sincT[cfg.nrbf, ecols] = fc

    ntab = cfg.ntab
    nsT = np.zeros((f, ntab), dtype=np.float32)
    nsT[:, :cfg.n] = node_scalar.T
    nv_tab = np.zeros((ntab, G), dtype=np.float32)
    nv_tab[:cfg.n] = node_vector.reshape(cfg.n, G)
    wr_ext = np.zeros((24, G), dtype=np.float32)
    wr_ext[:cfg.nrbf] = Wr
    wr_ext[cfg.nrbf] = br
    iota4 = np.tile(np.arange(128, dtype=np.float32), (128, 4))

    in_map = dict(
        nsT=nsT.astype(BF16),
        nv_tab=nv_tab.astype(BF16),
        w1=W1.astype(BF16),
        b1=b1.reshape(f, 1).astype(np.float32),
        w2=W2.astype(BF16),
        b2row=b2.reshape(1, G).astype(BF16),
        wr_ext=wr_ext.astype(BF16),
        iota4=iota4,
        ones1=np.ones((1, 128), dtype=BF16),
        srcidx=srccols,
        dstloc=dstcols,
        rhat=rhatcols.astype(BF16),
        sincT=sincT.astype(BF16),
        ns_base=ns_base,
        nv_base=nv_base,
    )
    return in_map, perm_nodes


def prep_all(cfg: Cfg, inputs):
    node_scalar = np.asarray(inputs["node_scalar"], np.float32)
    node_vector = np.asarray(inputs["node_vector"], np.float32)
    r_vec = np.asarray(inputs["r_vec"], np.float32)
    W1 = np.asarray(inputs["W1"], np.float32)
    b1 = np.asarray(inputs["b1"], np.float32)
    W2 = np.asarray(inputs["W2"], np.float32)
    b2 = np.asarray(inputs["b2"], np.float32)
    Wr = np.asarray(inputs["Wr"], np.float32)
    br = np.asarray(inputs["br"], np.float32)
    edge_idx = np.asarray(inputs["edge_idx"]).astype(np.int64)
    dst, src = edge_idx[:, 0], edge_idx[:, 1]

    edge_order = np.argsort(dst, kind="stable")
    counts = np.bincount(dst, minlength=cfg.n)
    node_ptr = np.zeros(cfg.n + 1, dtype=np.int64)
    np.cumsum(counts, out=node_ptr[1:])

    per_core_nodes = pack_blocks(dst, cfg)
    in_maps, perms = [], []
    for k in range(cfg.ncores):
        im, pm = prep_core_inputs(cfg, per_core_nodes[k], node_scalar,
                                  node_vector, r_vec, W1, b1, W2, b2, Wr, br,
                                  src, dst, edge_order, node_ptr)
        in_maps.append(im)
        perms.append(pm)
    return in_maps, perms


def assemble_outputs(cfg: Cfg, results, perms, node_scalar, node_vector):
    out_s = np.empty((cfg.n, cfg.f), dtype=np.float32)
    out_v = np.empty((cfg.n, cfg.f, 3), dtype=np.float32)
    for k in range(cfg.ncores):
        pm = perms[k]
        valid = pm >= 0
        out_s[pm[valid]] = results[k]["out_s"][valid]
        out_v[pm[valid]] = results[k]["out_v"][valid].reshape(-1, cfg.f, 3)
    return out_s, out_v


_CACHE = {}


def kernel(**inputs):
    from concourse.bass_utils import run_bass_kernel_spmd

    cfg = CFG
    in_maps, perms = prep_all(cfg, inputs)
    if "nc" not in _CACHE:
        _CACHE["nc"] = build_program(cfg)
    nc = _CACHE["nc"]
    br = run_bass_kernel_spmd(nc, in_maps, core_ids=list(range(cfg.ncores)))
    out_s, out_v = assemble_outputs(cfg, br.results, perms,
                                    inputs["node_scalar"],
                                    inputs["node_vector"])
    return out_s, out_v
